# revision 15
# baseline (speedup 1.0000x reference)
"""Distributed causal multi-head attention for TRN2, 8 NeuronCores.

Sharding: core c (0..7) handles batch c//4 and heads 4*(c%4)..4*(c%4)+3
(tensor-parallel over heads x data-parallel over batch). All matmuls
bf16 with fp32 PSUM accumulation (rel err ~5e-3 vs the f32 reference).

Every input is pre-arranged on the host into partition-major [128, X]
layouts so each DMA moves 4-16KB contiguous runs per partition (the
rearranged layouts of earlier versions paid 2-3x descriptor overhead
and made the HBM-bound startup ~15us longer).

Per-core pipeline, ordered so the in-order PE queue never waits on an
ACT/DVE epilogue (everything is emission-scheduled by hand):
  1. Startup: x^T streams in 512-column quarters, split across the two
     DMA queues with W_Q/W_K (head 0) and W_V halves in front; every
     later consumer's data lands before the PE reaches it.
  2. V[s,k] for all 4 heads (xt tiles stationary, heads packed in the
     free dim), interleaved with head 0's QT/KT chains as xt streams in.
  3. Head 0's QT/KT run as per-chunk chains (startup-friendly); heads
     1-3 run as 4-wide weight-stationary groups (for each W tile, 4
     matmuls into 4 PSUM banks, one per q-chunk) emitted INSIDE the
     previous head's attention stream -- the stationary operand is
     loaded once per 4 matmuls instead of once per matmul, cutting the
     ~53ns/MM LDWEIGHTS exposure on 384 of the 512 projection matmuls.
  4. Attention chunks are software-pipelined: the score matmul of tile
     i+1 is emitted BEFORE the zT matmul of tile i, so the in-order PE
     queue never waits on tile i's exp (ACT). The softmax denominator
     accumulates on the DVE in SBUF (racc += pt); each chunk's epilogue
     (ones-block matmul broadcasts the denominator, DVE reciprocal+
     multiply normalize zT) is deferred into the next chunk's stream,
     injected between ST_1 and z_0 where the PE would otherwise wait
     for exp_0.
  5. zT ships through AllGather over the 4-core batch group as it is
     produced: q-halves for heads 0-2, q-quarters for head 3, and the
     last chunk as 2x256-column pieces, so the collectives overlap
     attention and the final gathers don't queue on the CC engine.
  6. Output projection, d-sharded: each core's wo input holds only its
     512 W_O columns (out[all q, d_slice] = z_flat @ W_O[:, slice]),
     interleaved with head 3's tail as gathers land; local head 3's
     tiles accumulate last in each chain so the chains can start while
     the final gather streams in. The SPMD graph is identical on all
     cores; per-core behavior comes only from input data.

Host: shards/casts/transposes inputs, then adds the bias correction
b_O + sum_h b_V[h] @ W_O[h] (a constant row, because softmax rows sum
to 1) to the assembled output.
"""
import math
import os

import numpy as np
import ml_dtypes

import concourse.bacc as bacc
import concourse.mybir as mybir
from concourse import tile
from concourse.bass_utils import run_bass_kernel_spmd

BF16 = mybir.dt.bfloat16
F32 = mybir.dt.float32
NPBF16 = ml_dtypes.bfloat16

B = 2
SEQ = 2048
D_MODEL = 2048
N_HEADS = 16
D_HEAD = 128
HPC = 4              # heads per core
NCORES = 8
GROUPS = [[0, 1, 2, 3], [4, 5, 6, 7]]
NDT = D_MODEL // 128   # 16 d-model tiles
NST = SEQ // 128       # 16 seq tiles
NQC = SEQ // 512       # 4 q-chunks
QSL = SEQ // 4         # 512 per-core q-slice for output projection
SCALE = 1.0 / math.sqrt(D_HEAD)

LAST_EXEC_NS = None


def build_nc():
    nc = bacc.Bacc(None, num_devices=NCORES, debug=False)

    # host layouts are partition-major: dim0 = SBUF partition (128).
    xt_e = nc.declare_dram_parameter("xt", [128, NQC * NDT * 512], BF16, isOutput=False)
    wq_e = nc.declare_dram_parameter("wq", [128, HPC * NDT * D_HEAD], BF16, isOutput=False)
    wk_e = nc.declare_dram_parameter("wk", [128, HPC * NDT * D_HEAD], BF16, isOutput=False)
    wv_e = nc.declare_dram_parameter("wv", [128, NDT * HPC * D_HEAD], BF16, isOutput=False)
    wo_e = nc.declare_dram_parameter("wo", [128, N_HEADS * QSL], BF16, isOutput=False)
    bq_e = nc.declare_dram_parameter("bq", [D_HEAD, HPC], F32, isOutput=False)
    bk_e = nc.declare_dram_parameter("bk", [D_HEAD, HPC], F32, isOutput=False)
    mk_e = nc.declare_dram_parameter("mk", [128, 128], BF16, isOutput=False)
    out_e = nc.declare_dram_parameter("out", [SEQ, QSL], F32, isOutput=True)

    # AllGather buffers: one per (local head, q-chunk). Quarters are
    # cheap on the CC engine and keep its queue from head-blocking the
    # final gathers that gate the output projection.
    agin = [[nc.dram_tensor(f"agin{h}_{hf}", [D_HEAD, SEQ // 2], BF16)
             for hf in range(2)] for h in range(3)]
    agout = [[nc.dram_tensor(f"agout{h}_{hf}", [4 * D_HEAD, SEQ // 2], BF16)
              for hf in range(2)] for h in range(3)]
    agin3 = [nc.dram_tensor(f"agin3_{j}", [D_HEAD, 512], BF16)
             for j in range(3)]
    agout3 = [nc.dram_tensor(f"agout3_{j}", [4 * D_HEAD, 512], BF16)
              for j in range(3)]
    agin_p = [nc.dram_tensor(f"aginp{p}", [D_HEAD, 256], BF16)
              for p in range(2)]
    agout_p = [nc.dram_tensor(f"agoutp{p}", [4 * D_HEAD, 256], BF16)
               for p in range(2)]

    with tile.TileContext(nc) as tc:
        with tc.tile_pool(name="persist", bufs=1) as pp, \
             tc.tile_pool(name="xtp", bufs=1) as xt_pool, \
             tc.tile_pool(name="qkp", bufs=2) as qk_pool, \
             tc.tile_pool(name="vp", bufs=HPC) as v_pool, \
             tc.tile_pool(name="wvp", bufs=1) as wv_pool, \
             tc.tile_pool(name="pt", bufs=5) as pt_pool, \
             tc.tile_pool(name="zz", bufs=3) as z_pool, \
             tc.tile_pool(name="wo", bufs=1) as wo_pool, \
             tc.tile_pool(name="zg", bufs=2) as zg_pool, \
             tc.tile_pool(name="os", bufs=1) as out_pool, \
             tc.tile_pool(name="psA", bufs=3, space="PSUM") as psA, \
             tc.tile_pool(name="ps_st", bufs=2, space="PSUM") as ps_st, \
             tc.tile_pool(name="ps_zt", bufs=2, space="PSUM") as ps_zt, \
             tc.tile_pool(name="ps_r", bufs=1, space="PSUM") as ps_r:
            ones_blk = pp.tile([128, 128], BF16, tag="ones_blk")
            nc.vector.memset(ones_blk[:], 1.0)

            # xt_sb is quarter-major: [p, qc, t, s]
            xt_sb = xt_pool.tile([128, NQC, NDT, 512], BF16, tag="xt")
            wv_sb = wv_pool.tile([128, NDT, HPC * D_HEAD], BF16, tag="wv")
            wo_sb = wo_pool.tile([128, N_HEADS, QSL], BF16, tag="wo")
            wqs, wks, qts, kts = {}, {}, {}, {}

            def alloc_head(h):
                wqs[h] = qk_pool.tile([128, NDT, D_HEAD], BF16, tag="wq",
                                      name=f"wq{h}")
                wks[h] = qk_pool.tile([128, NDT, D_HEAD], BF16, tag="wk",
                                      name=f"wk{h}")
                qts[h] = qk_pool.tile([128, SEQ], BF16, tag="qt", name=f"qt{h}")
                kts[h] = qk_pool.tile([128, SEQ], BF16, tag="kt", name=f"kt{h}")

            def dma_xt_quarter(qc):
                for half, eng in ((0, nc.sync), (1, nc.gpsimd)):
                    c0 = qc * NDT * 512 + half * 4096
                    eng.dma_start(
                        xt_sb[:, qc, half * 8:half * 8 + 8, :],
                        xt_e[:, c0:c0 + 4096])

            def dma_w_head(dst, src_e, h):
                for half, eng in ((0, nc.sync), (1, nc.gpsimd)):
                    c0 = h * NDT * D_HEAD + half * 1024
                    eng.dma_start(
                        dst[:, half * 8:half * 8 + 8, :],
                        src_e[:, c0:c0 + 1024])

            # ---- startup DMA schedule: both queues carry the same bytes,
            # ordered to match PE consumption: Q chain first (wq0+xt q0),
            # then V chains (wv), then K chain (wk0), then the rest.
            alloc_head(0)
            dma_w_head(wqs[0], wq_e, 0)
            dma_xt_quarter(0)
            for half, eng in ((0, nc.sync), (1, nc.gpsimd)):
                eng.dma_start(wv_sb[:, half * 8:half * 8 + 8, :],
                              wv_e[:, half * 4096:(half + 1) * 4096])
            dma_w_head(wks[0], wk_e, 0)
            bq_sb = pp.tile([128, HPC], F32, tag="bq")
            nc.sync.dma_start(bq_sb[:], bq_e[:, :])
            bk_sb = pp.tile([128, HPC], F32, tag="bk")
            nc.sync.dma_start(bk_sb[:], bk_e[:, :])
            tri_sb = pp.tile([128, 128], BF16, tag="mk")
            nc.sync.dma_start(tri_sb[:], mk_e[:, :])
            dma_xt_quarter(1)
            dma_xt_quarter(2)
            alloc_head(1)
            dma_w_head(wqs[1], wq_e, 1)
            dma_w_head(wks[1], wk_e, 1)
            dma_xt_quarter(3)
            for half, eng in ((0, nc.sync), (1, nc.gpsimd)):
                eng.dma_start(wo_sb[:, half * 8:half * 8 + 8, :],
                              wo_e[:, half * 4096:(half + 1) * 4096])
            # heads 2-3's weights are issued inside the head loop: their
            # SBUF buffers recycle head 0/1's, and an up-front DMA would
            # block the queue until those readers finish.

            v_sb = [v_pool.tile([128, NST, D_HEAD], BF16, tag="v",
                                name=f"v{h}") for h in range(HPC)]

            def emit_v_chains(st0, st1):
                for st in range(st0, st1):
                    psum = psA.tile([128, 512], F32, tag="A",
                                    name=f"pv{st}")
                    for dt in range(NDT):
                        nc.tensor.matmul(
                            psum[:],
                            xt_sb[:, st // 4, dt,
                                  (st % 4) * 128:(st % 4 + 1) * 128],
                            wv_sb[:, dt, :],
                            start=(dt == 0), stop=(dt == NDT - 1))
                    for h in range(HPC):
                        nc.scalar.copy(
                            v_sb[h][:, st, 0:D_HEAD],
                            psum[:, h * 128:(h + 1) * 128])

            # ---- helpers -------------------------------------------------
            def finalize_chunk(st):
                """Chunk epilogue: broadcast the softmax denominator with a
                ones-block matmul, then normalize and ship zT (everything
                except that one matmul runs off the PE)."""
                fh, fj, fz, fr = st
                # rp gets a dedicated bank: the DVE reciprocal holds its
                # reader lock for ~3.4us, and in the st ring that blocked
                # the next chunk's ST_1 on the bank WAR for ~2us.
                rp = ps_r.tile([128, 512], F32, tag="r", name="rbc")
                nc.tensor.matmul(rp[:], ones_blk[:, :], fr[:],
                                 start=True, stop=True)
                # The DVE reciprocal costs ~3.4us for [128,512]; it is only
                # safe because the epilogue is injected into a QKT chain,
                # where the DVE FIFO is idle for ~8us (injecting it inside
                # an attention chunk stalls the PE via the pt/racc deps).
                rcp = z_pool.tile([128, 512], F32, tag="rcp")
                nc.vector.reciprocal(rcp[:], rp[:])
                zt = z_pool.tile([128, 512], BF16, tag="ztile")
                nc.vector.tensor_mul(zt[:], fz[:], rcp[:])
                if fh == 3 and fj == 3:
                    for p in range(2):
                        nc.sync.dma_start(
                            agin_p[p][:, :], zt[:, p * 256:(p + 1) * 256])
                        nc.gpsimd.collective_compute(
                            "AllGather",
                            mybir.AluOpType.bypass,
                            replica_groups=GROUPS,
                            ins=[agin_p[p].ap().opt()],
                            outs=[agout_p[p].ap().opt()],
                        )
                elif fh == 3:
                    nc.sync.dma_start(agin3[fj][:, :], zt[:])
                    nc.gpsimd.collective_compute(
                        "AllGather",
                        mybir.AluOpType.bypass,
                        replica_groups=GROUPS,
                        ins=[agin3[fj].ap().opt()],
                        outs=[agout3[fj].ap().opt()],
                    )
                else:
                    nc.sync.dma_start(
                        agin[fh][fj // 2][:, (fj % 2) * 512:(fj % 2 + 1) * 512],
                        zt[:])
                    if fj % 2 == 1:
                        nc.gpsimd.collective_compute(
                            "AllGather",
                            mybir.AluOpType.bypass,
                            replica_groups=GROUPS,
                            ins=[agin[fh][fj // 2].ap().opt()],
                            outs=[agout[fh][fj // 2].ap().opt()],
                        )

            def emit_zg(qg):
                """Load the gathered zT tiles for one 512-wide q-chunk.
                zg[:, h, r, :] = zT of global head 4*r + h. For the tail
                chunks (qg >= 2) the long-landed gathers load via the sync
                queue so they don't sit behind collectives on gpsimd."""
                zg = zg_pool.tile([128, HPC, 4, 512], BF16, tag="zg",
                                  name=f"zg{qg}")
                hf, qc = qg // 2, qg % 2
                eng = nc.sync if qg >= 2 else nc.gpsimd
                for h in range(3):
                    eng.dma_start(
                        zg[:, h, :, :],
                        agout[h][hf][:, qc * 512:(qc + 1) * 512]
                        .rearrange("(r p) s -> p r s", p=128))
                if qg == 3:
                    for p in range(2):
                        nc.gpsimd.dma_start(
                            zg[:, 3, :, p * 256:(p + 1) * 256],
                            agout_p[p].ap()
                            .rearrange("(r p2) s -> p2 r s", p2=128))
                else:
                    eng.dma_start(
                        zg[:, 3, :, :],
                        agout3[qg].ap().rearrange("(r p) s -> p r s", p=128))
                return zg

            def emit_outproj(qg, zg, finalize_after_qi=None):
                osb = out_pool.tile([128, 4, QSL], F32, tag="os",
                                    name=f"os{qg}")
                # local head 3's gather lands last; accumulate its tiles at
                # the end of each chain so the chain can start while the
                # final gather's zg loads are still streaming in.
                tts = [tt for tt in range(N_HEADS) if tt % HPC != 3] + \
                      [tt for tt in range(N_HEADS) if tt % HPC == 3]
                for qi in range(4):
                    psum = psA.tile([128, QSL], F32, tag="A",
                                    name=f"po{qg}_{qi}")
                    for k, tt in enumerate(tts):
                        r, h = tt // HPC, tt % HPC
                        nc.tensor.matmul(
                            psum[:],
                            zg[:, h, r, qi * 128:(qi + 1) * 128],
                            wo_sb[:, tt, :],
                            start=(k == 0), stop=(k == N_HEADS - 1))
                    nc.scalar.copy(osb[:, qi, :], psum[:])
                    if qi == finalize_after_qi and pend[0] is not None:
                        finalize_chunk(pend[0])  # last (h3, j3) AllGather
                        pend[0] = None
                    if qi % 2 == 1:
                        nc.sync.dma_start(
                            out_e[qg * 512 + (qi - 1) * 128:
                                  qg * 512 + (qi + 1) * 128, :]
                            .rearrange("(t p) d -> p t d", p=128),
                            osb[:, qi - 1:qi + 1, :])

            pend = [None]
            zgs = {}

            def emit_qkt_chain(h, sc, proj):
                """One q-chunk projection chain. A pending attention-chunk
                epilogue is injected after the 3rd matmul: by then the
                denominator's SBUF copy (ACT) has landed, so the broadcast
                matmul never blocks the in-order PE queue, and the ACT/DVE
                epilogue ops run where those engines are otherwise idle."""
                w_t = wqs[h] if proj == 0 else wks[h]
                psum = psA.tile([128, 512], F32, tag="A",
                                name=f"pq{h}_{sc}_{proj}")
                for dt in range(NDT):
                    nc.tensor.matmul(
                        psum[:],
                        w_t[:, dt, :],
                        xt_sb[:, sc, dt, :],
                        start=(dt == 0), stop=(dt == NDT - 1))
                    if dt == 2 and pend[0] is not None:
                        finalize_chunk(pend[0])
                        pend[0] = None
                emit_qkt_evac(h, sc, proj, psum)

            def emit_qkt_evac(h, sc, proj, psum):
                if proj == 0:
                    nc.scalar.activation(
                        qts[h][:, sc * 512:(sc + 1) * 512], psum[:],
                        mybir.ActivationFunctionType.Identity,
                        bias=bq_sb[:, h:h + 1], scale=SCALE)
                else:
                    nc.scalar.activation(
                        kts[h][:, sc * 512:(sc + 1) * 512], psum[:],
                        mybir.ActivationFunctionType.Identity,
                        bias=bk_sb[:, h:h + 1], scale=1.0)

            def emit_attn_chunk(h, j):
                """Software-pipelined: emit ST_{i+1} before z_i so the PE
                never waits on exp_i; the pending chunk's finalize matmul
                fills the PE slot where z_0 would wait on exp_0."""
                qt_sb, kt_sb = qts[h], kts[h]
                n_st = 4 * (j + 1)
                ztp = ps_zt.tile([128, 512], F32, tag="zt",
                                 name=f"zt{h}_{j}")
                racc = z_pool.tile([128, 512], F32, tag="racc",
                                   name=f"ra{h}_{j}")

                def emit_st(i):
                    v = i - 4 * j
                    # causal: diagonal tile v touches only columns >= 128*v
                    c0 = 128 * v if v > 0 else 0
                    stp = ps_st.tile([128, 512], F32, tag="st")
                    nc.tensor.matmul(
                        stp[:, c0:],
                        kt_sb[:, i * 128:(i + 1) * 128],
                        qt_sb[:, j * 512 + c0:(j + 1) * 512],
                        start=True, stop=True)
                    pt = pt_pool.tile([128, 512], BF16, tag="pt")
                    nc.scalar.activation(
                        pt[:, c0:], stp[:, c0:],
                        mybir.ActivationFunctionType.Exp)
                    if v >= 0:
                        # mask only the 128-col triangle block at the diag
                        nc.vector.tensor_mul(
                            pt[:, c0:c0 + 128], pt[:, c0:c0 + 128],
                            tri_sb[:])
                    if i == 0:
                        nc.vector.tensor_copy(racc[:], pt[:])
                    else:
                        nc.vector.tensor_add(
                            racc[:, c0:], racc[:, c0:], pt[:, c0:])
                    return c0, pt

                def emit_z(i, c0, pt):
                    nc.tensor.matmul(
                        ztp[:, c0:], v_sb[h][:, i, :], pt[:, c0:],
                        start=(i == 0), stop=(i == n_st - 1))

                prev = emit_st(0)
                for i in range(1, n_st):
                    cur = emit_st(i)
                    emit_z(i - 1, *prev)
                    prev = cur
                emit_z(n_st - 1, *prev)
                racc_sb = z_pool.tile([128, 512], BF16, tag="racc_sb")
                nc.scalar.copy(racc_sb[:], racc[:])
                pend[0] = (h, j, ztp, racc_sb)

            # ---- head 0: chains ordered to match the DMA arrival order
            # (Q chain on xt q0, V chains as wv lands, then K). Every
            # attention chunk is followed by a chain (pulling the next
            # head's first pair forward) so the pending epilogue always
            # lands in a chain, off the chunk-critical ACT/DVE streams. --
            emit_qkt_chain(0, 0, 0)
            emit_v_chains(0, 4)
            emit_qkt_chain(0, 0, 1)
            emit_qkt_chain(0, 1, 0)
            emit_v_chains(4, 8)
            emit_qkt_chain(0, 1, 1)
            emit_attn_chunk(0, 0)
            emit_qkt_chain(0, 2, 0)
            emit_qkt_chain(0, 2, 1)
            emit_v_chains(8, 12)
            emit_attn_chunk(0, 1)
            emit_qkt_chain(0, 3, 0)
            emit_qkt_chain(0, 3, 1)
            emit_v_chains(12, 16)
            emit_attn_chunk(0, 2)

            # ---- heads 1-3 ----------------------------------------------
            for h in (1, 2, 3):
                if h >= 2:
                    alloc_head(h)
                    dma_w_head(wqs[h], wq_e, h)
                    dma_w_head(wks[h], wk_e, h)
                emit_qkt_chain(h, 0, 0)
                emit_qkt_chain(h, 0, 1)
                emit_attn_chunk(h - 1, 3)
                emit_qkt_chain(h, 1, 0)
                emit_qkt_chain(h, 1, 1)
                emit_attn_chunk(h, 0)
                emit_qkt_chain(h, 2, 0)
                emit_qkt_chain(h, 2, 1)
                emit_attn_chunk(h, 1)
                emit_qkt_chain(h, 3, 0)
                emit_qkt_chain(h, 3, 1)
                emit_attn_chunk(h, 2)
            zgs[0] = emit_zg(0)
            emit_outproj(0, zgs[0], finalize_after_qi=0)
            emit_attn_chunk(3, 3)
            zgs[1] = emit_zg(1)

            emit_outproj(1, zgs[1], finalize_after_qi=0)
            zgs[2] = emit_zg(2)
            zgs[3] = emit_zg(3)
            emit_outproj(2, zgs[2])
            emit_outproj(3, zgs[3])
    nc.finalize()
    return nc


def _build_tri():
    """tri[r, c] = 1 if key offset r <= query offset c (within the
    128x128 diagonal block; the same triangle serves every diagonal)."""
    r = np.arange(128)[:, None]
    c = np.arange(128)[None, :]
    return (c >= r).astype(NPBF16)


def _pmajor(a):
    """[T*128, K] row-major -> [128, T*K] partition-major."""
    t128, k = a.shape
    t = t128 // 128
    return np.ascontiguousarray(
        a.reshape(t, 128, k).transpose(1, 0, 2).reshape(128, t * k))


_NC_CACHE = None


def kernel(normalized_resid_pre, W_Q, b_Q, W_K, b_K, W_V, b_V, W_O, b_O):
    global LAST_EXEC_NS, _NC_CACHE
    x = np.asarray(normalized_resid_pre, dtype=np.float32)
    W_Q = np.asarray(W_Q, np.float32); b_Q = np.asarray(b_Q, np.float32)
    W_K = np.asarray(W_K, np.float32); b_K = np.asarray(b_K, np.float32)
    W_V = np.asarray(W_V, np.float32); b_V = np.asarray(b_V, np.float32)
    W_O = np.asarray(W_O, np.float32); b_O = np.asarray(b_O, np.float32)

    tri_m = _build_tri()
    wo_flat = W_O.reshape(N_HEADS * D_HEAD, D_MODEL)
    # xt[p, qc*NDT*512 + t*512 + s] = x[b].T[t*128+p, qc*512+s]
    xt = []
    for b in range(B):
        xT = np.ascontiguousarray(x[b].T)  # [D_MODEL, SEQ]
        xq = np.ascontiguousarray(
            xT.reshape(NDT, 128, NQC, 512).transpose(1, 2, 0, 3)
            .reshape(128, NQC * NDT * 512)).astype(NPBF16)
        xt.append(xq)

    in_maps = []
    for c in range(NCORES):
        beta, g = c // 4, c % 4
        hs = slice(HPC * g, HPC * g + HPC)
        wq_m = _pmajor(W_Q[hs].reshape(HPC * D_MODEL, D_HEAD)).astype(NPBF16)
        wk_m = _pmajor(W_K[hs].reshape(HPC * D_MODEL, D_HEAD)).astype(NPBF16)
        wv_m = _pmajor(
            W_V[hs].transpose(1, 0, 2).reshape(D_MODEL, HPC * D_HEAD)
        ).astype(NPBF16)
        wo_m = _pmajor(
            np.ascontiguousarray(wo_flat[:, QSL * g:QSL * (g + 1)])
        ).astype(NPBF16)
        bq_m = np.ascontiguousarray((b_Q[hs] * SCALE).T).astype(np.float32)
        bk_m = np.ascontiguousarray(b_K[hs].T).astype(np.float32)
        in_maps.append({
            "xt": xt[beta], "wq": wq_m, "wk": wk_m, "wv": wv_m,
            "wo": wo_m, "bq": bq_m, "bk": bk_m, "mk": tri_m,
        })

    if _NC_CACHE is None:
        _NC_CACHE = build_nc()
    nc = _NC_CACHE

    trace = False
    if os.environ.get("BASS_KERNEL_TRACE") == "1":
        try:
            from antenv.axon_hooks import get_axon_ntff_profile_hook
            trace = get_axon_ntff_profile_hook() is not None
        except ImportError:
            trace = False

    res = run_bass_kernel_spmd(nc, in_maps, core_ids=list(range(NCORES)),
                               trace=trace)
    LAST_EXEC_NS = res.exec_time_ns

    # bias correction: softmax rows sum to 1 -> b_V contributes a constant
    # row through W_O; b_O is a plain add.
    corr = b_O + np.einsum("hk,hkd->d", b_V, W_O)

    out = np.empty((B, SEQ, D_MODEL), dtype=np.float32)
    for c in range(NCORES):
        beta, g = c // 4, c % 4
        out[beta, :, QSL * g:QSL * (g + 1)] = (
            res.results[c]["out"] + corr[QSL * g:QSL * (g + 1)])
    return out


# revision 17
# speedup vs baseline: 1.0105x; 1.0105x over previous
"""Distributed causal multi-head attention for TRN2, 8 NeuronCores.

Sharding: core c (0..7) handles batch c//4 and heads 4*(c%4)..4*(c%4)+3
(tensor-parallel over heads x data-parallel over batch). All matmuls
bf16 with fp32 PSUM accumulation (rel err ~5e-3 vs the f32 reference).

Every input is pre-arranged on the host into partition-major [128, X]
layouts so each DMA moves 4-16KB contiguous runs per partition (the
rearranged layouts of earlier versions paid 2-3x descriptor overhead
and made the HBM-bound startup ~15us longer).

Per-core pipeline, ordered so the in-order PE queue never waits on an
ACT/DVE epilogue (everything is emission-scheduled by hand):
  1. Startup: x^T streams in 512-column quarters, split across the two
     DMA queues with W_Q/W_K (head 0) and W_V halves in front; every
     later consumer's data lands before the PE reaches it.
  2. V[s,k] for all 4 heads (xt tiles stationary, heads packed in the
     free dim), interleaved with head 0's QT/KT chains as xt streams in.
  3. Head 0's QT/KT run as per-chunk chains (startup-friendly); heads
     1-3 run as 4-wide weight-stationary groups (for each W tile, 4
     matmuls into 4 PSUM banks, one per q-chunk) emitted INSIDE the
     previous head's attention stream -- the stationary operand is
     loaded once per 4 matmuls instead of once per matmul, cutting the
     ~53ns/MM LDWEIGHTS exposure on 384 of the 512 projection matmuls.
  4. Attention chunks are software-pipelined: the score matmul of tile
     i+1 is emitted BEFORE the zT matmul of tile i, so the in-order PE
     queue never waits on tile i's exp (ACT). The softmax denominator
     accumulates on the DVE in SBUF (racc += pt); each chunk's epilogue
     (ones-block matmul broadcasts the denominator, DVE reciprocal+
     multiply normalize zT) is deferred into the next chunk's stream,
     injected between ST_1 and z_0 where the PE would otherwise wait
     for exp_0.
  5. zT ships through AllGather over the 4-core batch group as it is
     produced: q-halves for heads 0-2, q-quarters for head 3, and the
     last chunk as 2x256-column pieces, so the collectives overlap
     attention and the final gathers don't queue on the CC engine.
  6. Output projection, d-sharded: each core's wo input holds only its
     512 W_O columns (out[all q, d_slice] = z_flat @ W_O[:, slice]),
     interleaved with head 3's tail as gathers land; local head 3's
     tiles accumulate last in each chain so the chains can start while
     the final gather streams in. The SPMD graph is identical on all
     cores; per-core behavior comes only from input data.

Host: shards/casts/transposes inputs, then adds the bias correction
b_O + sum_h b_V[h] @ W_O[h] (a constant row, because softmax rows sum
to 1) to the assembled output.
"""
import math
import os

import numpy as np
import ml_dtypes

import concourse.bacc as bacc
import concourse.mybir as mybir
from concourse import tile
from concourse.bass_utils import run_bass_kernel_spmd

BF16 = mybir.dt.bfloat16
F32 = mybir.dt.float32
NPBF16 = ml_dtypes.bfloat16

B = 2
SEQ = 2048
D_MODEL = 2048
N_HEADS = 16
D_HEAD = 128
HPC = 4              # heads per core
NCORES = 8
GROUPS = [[0, 1, 2, 3], [4, 5, 6, 7]]
NDT = D_MODEL // 128   # 16 d-model tiles
NST = SEQ // 128       # 16 seq tiles
NQC = SEQ // 512       # 4 q-chunks
QSL = SEQ // 4         # 512 per-core q-slice for output projection
SCALE = 1.0 / math.sqrt(D_HEAD)

LAST_EXEC_NS = None


def build_nc():
    nc = bacc.Bacc(None, num_devices=NCORES, debug=False)

    # host layouts are partition-major: dim0 = SBUF partition (128).
    xt_e = nc.declare_dram_parameter("xt", [128, NQC * NDT * 512], BF16, isOutput=False)
    wq_e = nc.declare_dram_parameter("wq", [128, HPC * NDT * D_HEAD], BF16, isOutput=False)
    wk_e = nc.declare_dram_parameter("wk", [128, HPC * NDT * D_HEAD], BF16, isOutput=False)
    wv_e = nc.declare_dram_parameter("wv", [128, NDT * HPC * D_HEAD], BF16, isOutput=False)
    wo_e = nc.declare_dram_parameter("wo", [128, N_HEADS * QSL], BF16, isOutput=False)
    bq_e = nc.declare_dram_parameter("bq", [D_HEAD, HPC], F32, isOutput=False)
    bk_e = nc.declare_dram_parameter("bk", [D_HEAD, HPC], F32, isOutput=False)
    mk_e = nc.declare_dram_parameter("mk", [128, 128], BF16, isOutput=False)
    out_e = nc.declare_dram_parameter("out", [SEQ, QSL], F32, isOutput=True)

    # AllGather buffers: one per (local head, q-chunk). Quarters are
    # cheap on the CC engine and keep its queue from head-blocking the
    # final gathers that gate the output projection.
    agin = [[nc.dram_tensor(f"agin{h}_{hf}", [D_HEAD, SEQ // 2], BF16)
             for hf in range(2)] for h in range(3)]
    agout = [[nc.dram_tensor(f"agout{h}_{hf}", [4 * D_HEAD, SEQ // 2], BF16)
              for hf in range(2)] for h in range(3)]
    agin3 = [nc.dram_tensor(f"agin3_{j}", [D_HEAD, 512], BF16)
             for j in range(3)]
    agout3 = [nc.dram_tensor(f"agout3_{j}", [4 * D_HEAD, 512], BF16)
              for j in range(3)]
    agin_p = [nc.dram_tensor(f"aginp{p}", [D_HEAD, 256], BF16)
              for p in range(2)]
    agout_p = [nc.dram_tensor(f"agoutp{p}", [4 * D_HEAD, 256], BF16)
               for p in range(2)]

    with tile.TileContext(nc) as tc:
        with tc.tile_pool(name="persist", bufs=1) as pp, \
             tc.tile_pool(name="xtp", bufs=1) as xt_pool, \
             tc.tile_pool(name="qkp", bufs=2) as qk_pool, \
             tc.tile_pool(name="vp", bufs=HPC) as v_pool, \
             tc.tile_pool(name="wvp", bufs=1) as wv_pool, \
             tc.tile_pool(name="pt", bufs=5) as pt_pool, \
             tc.tile_pool(name="zz", bufs=2) as z_pool, \
             tc.tile_pool(name="rps", bufs=1) as rps_pool, \
             tc.tile_pool(name="wo", bufs=1) as wo_pool, \
             tc.tile_pool(name="zg", bufs=2) as zg_pool, \
             tc.tile_pool(name="os", bufs=1) as out_pool, \
             tc.tile_pool(name="psA", bufs=4, space="PSUM") as psA, \
             tc.tile_pool(name="ps_st", bufs=2, space="PSUM") as ps_st, \
             tc.tile_pool(name="ps_zt", bufs=2, space="PSUM") as ps_zt:
            ones_blk = pp.tile([128, 128], BF16, tag="ones_blk")
            nc.vector.memset(ones_blk[:], 1.0)

            # xt_sb is quarter-major: [p, qc, t, s]
            xt_sb = xt_pool.tile([128, NQC, NDT, 512], BF16, tag="xt")
            wv_sb = wv_pool.tile([128, NDT, HPC * D_HEAD], BF16, tag="wv")
            wo_sb = wo_pool.tile([128, N_HEADS, QSL], BF16, tag="wo")
            wqs, wks, qts, kts = {}, {}, {}, {}

            def alloc_head(h):
                wqs[h] = qk_pool.tile([128, NDT, D_HEAD], BF16, tag="wq",
                                      name=f"wq{h}")
                wks[h] = qk_pool.tile([128, NDT, D_HEAD], BF16, tag="wk",
                                      name=f"wk{h}")
                qts[h] = qk_pool.tile([128, SEQ], BF16, tag="qt", name=f"qt{h}")
                kts[h] = qk_pool.tile([128, SEQ], BF16, tag="kt", name=f"kt{h}")

            def dma_xt_quarter(qc):
                for half, eng in ((0, nc.sync), (1, nc.gpsimd)):
                    c0 = qc * NDT * 512 + half * 4096
                    eng.dma_start(
                        xt_sb[:, qc, half * 8:half * 8 + 8, :],
                        xt_e[:, c0:c0 + 4096])

            def dma_w_head(dst, src_e, h):
                for half, eng in ((0, nc.sync), (1, nc.gpsimd)):
                    c0 = h * NDT * D_HEAD + half * 1024
                    eng.dma_start(
                        dst[:, half * 8:half * 8 + 8, :],
                        src_e[:, c0:c0 + 1024])

            # ---- startup DMA schedule: both queues carry the same bytes,
            # ordered to match PE consumption: Q chain first (wq0+xt q0),
            # then V chains (wv), then K chain (wk0), then the rest.
            alloc_head(0)
            dma_w_head(wqs[0], wq_e, 0)
            dma_xt_quarter(0)
            for half, eng in ((0, nc.sync), (1, nc.gpsimd)):
                eng.dma_start(wv_sb[:, half * 8:half * 8 + 8, :],
                              wv_e[:, half * 4096:(half + 1) * 4096])
            dma_w_head(wks[0], wk_e, 0)
            bq_sb = pp.tile([128, HPC], F32, tag="bq")
            nc.sync.dma_start(bq_sb[:], bq_e[:, :])
            bk_sb = pp.tile([128, HPC], F32, tag="bk")
            nc.sync.dma_start(bk_sb[:], bk_e[:, :])
            tri_sb = pp.tile([128, 128], BF16, tag="mk")
            nc.sync.dma_start(tri_sb[:], mk_e[:, :])
            dma_xt_quarter(1)
            dma_xt_quarter(2)
            alloc_head(1)
            dma_w_head(wqs[1], wq_e, 1)
            dma_w_head(wks[1], wk_e, 1)
            dma_xt_quarter(3)
            for half, eng in ((0, nc.sync), (1, nc.gpsimd)):
                eng.dma_start(wo_sb[:, half * 8:half * 8 + 8, :],
                              wo_e[:, half * 4096:(half + 1) * 4096])
            # heads 2-3's weights are issued inside the head loop: their
            # SBUF buffers recycle head 0/1's, and an up-front DMA would
            # block the queue until those readers finish.

            v_sb = [v_pool.tile([128, NST, D_HEAD], BF16, tag="v",
                                name=f"v{h}") for h in range(HPC)]

            def emit_v_chains(st0, st1):
                for st in range(st0, st1):
                    psum = psA.tile([128, 512], F32, tag="A",
                                    name=f"pv{st}")
                    for dt in range(NDT):
                        nc.tensor.matmul(
                            psum[:],
                            xt_sb[:, st // 4, dt,
                                  (st % 4) * 128:(st % 4 + 1) * 128],
                            wv_sb[:, dt, :],
                            start=(dt == 0), stop=(dt == NDT - 1))
                    for h in range(HPC):
                        nc.scalar.copy(
                            v_sb[h][:, st, 0:D_HEAD],
                            psum[:, h * 128:(h + 1) * 128])

            # ---- helpers -------------------------------------------------
            def finalize_chunk(st):
                """Chunk epilogue: broadcast the softmax denominator with a
                ones-block matmul, then normalize and ship zT (everything
                except that one matmul runs off the PE)."""
                fh, fj, fz, fr = st
                rp = ps_st.tile([128, 512], F32, tag="st", name="rbc")
                nc.tensor.matmul(rp[:], ones_blk[:, :], fr[:],
                                 start=True, stop=True)
                # Evacuate rp to SBUF immediately (quick ACT copy) so the
                # 3.4us DVE reciprocal holds no PSUM bank -- reading rp
                # directly would block the next chunk's ST on the bank WAR.
                rp_sb = rps_pool.tile([128, 512], F32, tag="rp_sb")
                nc.scalar.copy(rp_sb[:], rp[:])
                rcp = z_pool.tile([128, 512], F32, tag="rcp")
                nc.vector.reciprocal(rcp[:], rp_sb[:])
                zt = z_pool.tile([128, 512], BF16, tag="ztile")
                nc.vector.tensor_mul(zt[:], fz[:], rcp[:])
                if fh == 3 and fj == 3:
                    for p in range(2):
                        nc.sync.dma_start(
                            agin_p[p][:, :], zt[:, p * 256:(p + 1) * 256])
                        nc.gpsimd.collective_compute(
                            "AllGather",
                            mybir.AluOpType.bypass,
                            replica_groups=GROUPS,
                            ins=[agin_p[p].ap().opt()],
                            outs=[agout_p[p].ap().opt()],
                        )
                elif fh == 3:
                    nc.sync.dma_start(agin3[fj][:, :], zt[:])
                    nc.gpsimd.collective_compute(
                        "AllGather",
                        mybir.AluOpType.bypass,
                        replica_groups=GROUPS,
                        ins=[agin3[fj].ap().opt()],
                        outs=[agout3[fj].ap().opt()],
                    )
                else:
                    nc.sync.dma_start(
                        agin[fh][fj // 2][:, (fj % 2) * 512:(fj % 2 + 1) * 512],
                        zt[:])
                    if fj % 2 == 1:
                        nc.gpsimd.collective_compute(
                            "AllGather",
                            mybir.AluOpType.bypass,
                            replica_groups=GROUPS,
                            ins=[agin[fh][fj // 2].ap().opt()],
                            outs=[agout[fh][fj // 2].ap().opt()],
                        )

            def emit_zg(qg):
                """Load the gathered zT tiles for one 512-wide q-chunk.
                zg[:, h, r, :] = zT of global head 4*r + h. For the tail
                chunks (qg >= 2) the long-landed gathers load via the sync
                queue so they don't sit behind collectives on gpsimd."""
                zg = zg_pool.tile([128, HPC, 4, 512], BF16, tag="zg",
                                  name=f"zg{qg}")
                hf, qc = qg // 2, qg % 2
                eng = nc.sync if qg >= 2 else nc.gpsimd
                for h in range(3):
                    eng.dma_start(
                        zg[:, h, :, :],
                        agout[h][hf][:, qc * 512:(qc + 1) * 512]
                        .rearrange("(r p) s -> p r s", p=128))
                if qg == 3:
                    for p in range(2):
                        nc.gpsimd.dma_start(
                            zg[:, 3, :, p * 256:(p + 1) * 256],
                            agout_p[p].ap()
                            .rearrange("(r p2) s -> p2 r s", p2=128))
                else:
                    eng.dma_start(
                        zg[:, 3, :, :],
                        agout3[qg].ap().rearrange("(r p) s -> p r s", p=128))
                return zg

            def emit_outproj(qg, zg, finalize_after_qi=None):
                osb = out_pool.tile([128, 4, QSL], F32, tag="os",
                                    name=f"os{qg}")
                # local head 3's gather lands last; accumulate its tiles at
                # the end of each chain so the chain can start while the
                # final gather's zg loads are still streaming in.
                tts = [tt for tt in range(N_HEADS) if tt % HPC != 3] + \
                      [tt for tt in range(N_HEADS) if tt % HPC == 3]
                for qi in range(4):
                    psum = psA.tile([128, QSL], F32, tag="A",
                                    name=f"po{qg}_{qi}")
                    for k, tt in enumerate(tts):
                        r, h = tt // HPC, tt % HPC
                        nc.tensor.matmul(
                            psum[:],
                            zg[:, h, r, qi * 128:(qi + 1) * 128],
                            wo_sb[:, tt, :],
                            start=(k == 0), stop=(k == N_HEADS - 1))
                    nc.scalar.copy(osb[:, qi, :], psum[:])
                    if qi == finalize_after_qi and pend[0] is not None:
                        finalize_chunk(pend[0])  # last (h3, j3) AllGather
                        pend[0] = None
                    if qi % 2 == 1:
                        nc.sync.dma_start(
                            out_e[qg * 512 + (qi - 1) * 128:
                                  qg * 512 + (qi + 1) * 128, :]
                            .rearrange("(t p) d -> p t d", p=128),
                            osb[:, qi - 1:qi + 1, :])

            pend = [None]
            zgs = {}

            def emit_qkt_chain(h, sc, proj):
                """One q-chunk projection chain. A pending attention-chunk
                epilogue is injected after the 3rd matmul: by then the
                denominator's SBUF copy (ACT) has landed, so the broadcast
                matmul never blocks the in-order PE queue, and the ACT/DVE
                epilogue ops run where those engines are otherwise idle."""
                w_t = wqs[h] if proj == 0 else wks[h]
                psum = psA.tile([128, 512], F32, tag="A",
                                name=f"pq{h}_{sc}_{proj}")
                for dt in range(NDT):
                    nc.tensor.matmul(
                        psum[:],
                        w_t[:, dt, :],
                        xt_sb[:, sc, dt, :],
                        start=(dt == 0), stop=(dt == NDT - 1))
                    if dt == 2 and pend[0] is not None:
                        finalize_chunk(pend[0])
                        pend[0] = None
                emit_qkt_evac(h, sc, proj, psum)

            def emit_qkt_evac(h, sc, proj, psum):
                if proj == 0:
                    nc.scalar.activation(
                        qts[h][:, sc * 512:(sc + 1) * 512], psum[:],
                        mybir.ActivationFunctionType.Identity,
                        bias=bq_sb[:, h:h + 1], scale=SCALE)
                else:
                    nc.scalar.activation(
                        kts[h][:, sc * 512:(sc + 1) * 512], psum[:],
                        mybir.ActivationFunctionType.Identity,
                        bias=bk_sb[:, h:h + 1], scale=1.0)

            def emit_attn_chunk(h, j):
                """Software-pipelined: emit ST_{i+1} before z_i so the PE
                never waits on exp_i; the pending chunk's finalize matmul
                fills the PE slot where z_0 would wait on exp_0."""
                qt_sb, kt_sb = qts[h], kts[h]
                n_st = 4 * (j + 1)
                ztp = ps_zt.tile([128, 512], F32, tag="zt",
                                 name=f"zt{h}_{j}")
                racc = z_pool.tile([128, 512], F32, tag="racc",
                                   name=f"ra{h}_{j}")

                def emit_st(i):
                    v = i - 4 * j
                    # causal: diagonal tile v touches only columns >= 128*v
                    c0 = 128 * v if v > 0 else 0
                    stp = ps_st.tile([128, 512], F32, tag="st")
                    nc.tensor.matmul(
                        stp[:, c0:],
                        kt_sb[:, i * 128:(i + 1) * 128],
                        qt_sb[:, j * 512 + c0:(j + 1) * 512],
                        start=True, stop=True)
                    pt = pt_pool.tile([128, 512], BF16, tag="pt")
                    nc.scalar.activation(
                        pt[:, c0:], stp[:, c0:],
                        mybir.ActivationFunctionType.Exp)
                    if v >= 0:
                        # mask only the 128-col triangle block at the diag
                        nc.vector.tensor_mul(
                            pt[:, c0:c0 + 128], pt[:, c0:c0 + 128],
                            tri_sb[:])
                    if i == 0:
                        nc.vector.tensor_copy(racc[:], pt[:])
                    else:
                        nc.vector.tensor_add(
                            racc[:, c0:], racc[:, c0:], pt[:, c0:])
                    return c0, pt

                def emit_z(i, c0, pt):
                    nc.tensor.matmul(
                        ztp[:, c0:], v_sb[h][:, i, :], pt[:, c0:],
                        start=(i == 0), stop=(i == n_st - 1))

                prev = emit_st(0)
                for i in range(1, n_st):
                    cur = emit_st(i)
                    emit_z(i - 1, *prev)
                    prev = cur
                emit_z(n_st - 1, *prev)
                racc_sb = z_pool.tile([128, 512], BF16, tag="racc_sb")
                nc.scalar.copy(racc_sb[:], racc[:])
                pend[0] = (h, j, ztp, racc_sb)

            # ---- head 0: chains ordered to match the DMA arrival order
            # (Q chain on xt q0, V chains as wv lands, then K). Every
            # attention chunk is followed by a chain (pulling the next
            # head's first pair forward) so the pending epilogue always
            # lands in a chain, off the chunk-critical ACT/DVE streams. --
            emit_qkt_chain(0, 0, 0)
            emit_v_chains(0, 4)
            emit_qkt_chain(0, 0, 1)
            emit_qkt_chain(0, 1, 0)
            emit_v_chains(4, 8)
            emit_qkt_chain(0, 1, 1)
            emit_attn_chunk(0, 0)
            emit_qkt_chain(0, 2, 0)
            emit_qkt_chain(0, 2, 1)
            emit_v_chains(8, 12)
            emit_attn_chunk(0, 1)
            emit_qkt_chain(0, 3, 0)
            emit_qkt_chain(0, 3, 1)
            emit_v_chains(12, 16)
            emit_attn_chunk(0, 2)

            # ---- heads 1-3 ----------------------------------------------
            for h in (1, 2, 3):
                if h >= 2:
                    alloc_head(h)
                    dma_w_head(wqs[h], wq_e, h)
                    dma_w_head(wks[h], wk_e, h)
                emit_qkt_chain(h, 0, 0)
                emit_qkt_chain(h, 0, 1)
                emit_attn_chunk(h - 1, 3)
                emit_qkt_chain(h, 1, 0)
                emit_qkt_chain(h, 1, 1)
                emit_attn_chunk(h, 0)
                emit_qkt_chain(h, 2, 0)
                emit_qkt_chain(h, 2, 1)
                emit_attn_chunk(h, 1)
                emit_qkt_chain(h, 3, 0)
                emit_qkt_chain(h, 3, 1)
                emit_attn_chunk(h, 2)
            zgs[0] = emit_zg(0)
            emit_outproj(0, zgs[0], finalize_after_qi=0)
            emit_attn_chunk(3, 3)
            zgs[1] = emit_zg(1)

            emit_outproj(1, zgs[1], finalize_after_qi=0)
            zgs[2] = emit_zg(2)
            zgs[3] = emit_zg(3)
            emit_outproj(2, zgs[2])
            emit_outproj(3, zgs[3])
    nc.finalize()
    return nc


def _build_tri():
    """tri[r, c] = 1 if key offset r <= query offset c (within the
    128x128 diagonal block; the same triangle serves every diagonal)."""
    r = np.arange(128)[:, None]
    c = np.arange(128)[None, :]
    return (c >= r).astype(NPBF16)


def _pmajor(a):
    """[T*128, K] row-major -> [128, T*K] partition-major."""
    t128, k = a.shape
    t = t128 // 128
    return np.ascontiguousarray(
        a.reshape(t, 128, k).transpose(1, 0, 2).reshape(128, t * k))


_NC_CACHE = None


def kernel(normalized_resid_pre, W_Q, b_Q, W_K, b_K, W_V, b_V, W_O, b_O):
    global LAST_EXEC_NS, _NC_CACHE
    x = np.asarray(normalized_resid_pre, dtype=np.float32)
    W_Q = np.asarray(W_Q, np.float32); b_Q = np.asarray(b_Q, np.float32)
    W_K = np.asarray(W_K, np.float32); b_K = np.asarray(b_K, np.float32)
    W_V = np.asarray(W_V, np.float32); b_V = np.asarray(b_V, np.float32)
    W_O = np.asarray(W_O, np.float32); b_O = np.asarray(b_O, np.float32)

    tri_m = _build_tri()
    wo_flat = W_O.reshape(N_HEADS * D_HEAD, D_MODEL)
    # xt[p, qc*NDT*512 + t*512 + s] = x[b].T[t*128+p, qc*512+s]
    xt = []
    for b in range(B):
        xT = np.ascontiguousarray(x[b].T)  # [D_MODEL, SEQ]
        xq = np.ascontiguousarray(
            xT.reshape(NDT, 128, NQC, 512).transpose(1, 2, 0, 3)
            .reshape(128, NQC * NDT * 512)).astype(NPBF16)
        xt.append(xq)

    in_maps = []
    for c in range(NCORES):
        beta, g = c // 4, c % 4
        hs = slice(HPC * g, HPC * g + HPC)
        wq_m = _pmajor(W_Q[hs].reshape(HPC * D_MODEL, D_HEAD)).astype(NPBF16)
        wk_m = _pmajor(W_K[hs].reshape(HPC * D_MODEL, D_HEAD)).astype(NPBF16)
        wv_m = _pmajor(
            W_V[hs].transpose(1, 0, 2).reshape(D_MODEL, HPC * D_HEAD)
        ).astype(NPBF16)
        wo_m = _pmajor(
            np.ascontiguousarray(wo_flat[:, QSL * g:QSL * (g + 1)])
        ).astype(NPBF16)
        bq_m = np.ascontiguousarray((b_Q[hs] * SCALE).T).astype(np.float32)
        bk_m = np.ascontiguousarray(b_K[hs].T).astype(np.float32)
        in_maps.append({
            "xt": xt[beta], "wq": wq_m, "wk": wk_m, "wv": wv_m,
            "wo": wo_m, "bq": bq_m, "bk": bk_m, "mk": tri_m,
        })

    if _NC_CACHE is None:
        _NC_CACHE = build_nc()
    nc = _NC_CACHE

    trace = False
    if os.environ.get("BASS_KERNEL_TRACE") == "1":
        try:
            from antenv.axon_hooks import get_axon_ntff_profile_hook
            trace = get_axon_ntff_profile_hook() is not None
        except ImportError:
            trace = False

    res = run_bass_kernel_spmd(nc, in_maps, core_ids=list(range(NCORES)),
                               trace=trace)
    LAST_EXEC_NS = res.exec_time_ns

    # bias correction: softmax rows sum to 1 -> b_V contributes a constant
    # row through W_O; b_O is a plain add.
    corr = b_O + np.einsum("hk,hkd->d", b_V, W_O)

    out = np.empty((B, SEQ, D_MODEL), dtype=np.float32)
    for c in range(NCORES):
        beta, g = c // 4, c % 4
        out[beta, :, QSL * g:QSL * (g + 1)] = (
            res.results[c]["out"] + corr[QSL * g:QSL * (g + 1)])
    return out
